# revision 5
# baseline (speedup 1.0000x reference)
"""AttnBlock (GroupNorm -> QKV -> full attention -> proj + residual) on 8
Trainium2 NeuronCores, data-parallel over batch (b=8, one sample per core).

fp8 (e4m3, max 240) DoubleRow pipeline. Per core:
  h = GroupNorm(x) written directly as fp8 pairs (scale Sh folded into
  gamma/beta); x streamed in 512-col chunks with stats (sum/sumsq via
  DVE bn_stats + ACT Ident/Square accum) overlapping the DMA. Weights
  folded on host: A = (wq.T wk)/sqrt(c), WPV = wp wv. u = A.T h + g.
  scoresT[j,i] = h.T u computed DIRECTLY transposed (no PE transposes,
  no PSUM->SBUF copies): exp writes attT fp8 straight to SBUF with a
  global shift (exact softmax identity). vp and row-sum matmuls hide
  under the exp tail. outT[i,c] = attT.T @ vp is i-major so 1/rowsum is
  a per-partition activation scale; tiny PE transposes give rec. Final
  bf16 PE transpose back to c-major + fused (psum+bp)+x residual on
  DVE. All big matmuls fp8 DoubleRow (2 K-tiles per instruction).
"""

import functools

import numpy as np

B = 8
C = 512
W = 2048
G = 32
EPS = 1e-6
P = 128
CT = C // P          # 4 channel tiles
CP = CT // 2         # 2 channel-tile pairs
NW = W // 512        # 4 w-chunks of 512
IT = W // P          # 16 i-tiles
JP = IT // 2         # 8 j-tile pairs

SA = 256.0           # fp8 scale for A
SH = 16.0            # fp8 scale for h
SU = 8.0             # fp8 scale for u
SVP = 4.0            # fp8 scale for vp
SWPV = 64.0          # fp8 scale for WPV weights
ASHIFT = 1.5         # global score shift before exp (cancels in softmax)

TRACE = False
LAST_EXEC_NS = None
LAST_TRACE_PATH = None


def _build_nc():
    import concourse.mybir as mybir
    import concourse.tile as tile
    from concourse import bacc
    from concourse.masks import make_identity

    f32 = mybir.dt.float32
    f8 = mybir.dt.float8e4
    bf16 = mybir.dt.bfloat16
    Ident = mybir.ActivationFunctionType.Identity
    Exp = mybir.ActivationFunctionType.Exp
    Sqrt = mybir.ActivationFunctionType.Sqrt
    Square = mybir.ActivationFunctionType.Square
    mult = mybir.AluOpType.mult
    add = mybir.AluOpType.add
    subtract = mybir.AluOpType.subtract
    DR = mybir.MatmulPerfMode.DoubleRow

    nc = bacc.Bacc()

    x_d = nc.declare_dram_parameter("x", [C, W], f32, isOutput=False)
    # fp8 weights, pair-major [P, CP*2*C]: [p, cp, s, co] = w[(2cp+s)*128+p, co]
    a8_d = nc.declare_dram_parameter("a8", [P, CP * 2 * C], f8, isOutput=False)
    wpv8_d = nc.declare_dram_parameter("wpv8", [P, CP * 2 * C], f8, isOutput=False)
    # packed constants: [0:512] group-avg selector S, [512:1024] selector-back
    # ST, then gSu, bp_eff, gam*SH, bet*SH (CT cols each).
    aux_d = nc.declare_dram_parameter("aux", [P, 1040], f32, isOutput=False)
    out_d = nc.declare_dram_parameter("out", [C, W], f32, isOutput=True)

    with tile.TileContext(nc) as tc:
        with (
            tc.tile_pool(name="singles", bufs=1) as singles,
            tc.tile_pool(name="w8", bufs=1) as w8p,
            tc.tile_pool(name="xp", bufs=1) as xp,
            tc.tile_pool(name="h8p", bufs=1) as h8p,
            tc.tile_pool(name="att8p", bufs=1) as att8p,
            tc.tile_pool(name="outp", bufs=1) as outp,
            tc.tile_pool(name="gn", bufs=2) as gnp,
        ):
            # ---- persistent SBUF ----
            a8_sb = w8p.tile([P, CP, 2, C], f8, name="a8_sb")
            wpv8_sb = w8p.tile([P, CP, 2, C], f8, name="wpv8_sb")
            x_sb = [xp.tile([P, W], f32, name=f"x{t}") for t in range(CT)]
            h8 = h8p.tile([P, CP, 2, W], f8, name="h8")
            u8 = h8p.tile([P, CP, 2, W], f8, name="u8")
            vp8 = [h8p.tile([P, 2, C], f8, name=f"vp8_{jp}") for jp in range(JP)]
            att8 = [att8p.tile([P, 2, W], f8, name=f"att8_{jp}") for jp in range(JP)]
            outT = [outp.tile([P, 512], bf16, name=f"outT{it}") for it in range(IT)]
            scratch = singles.tile([P, W], f32, name="scratch")

            ident = singles.tile([P, P], f32, name="ident")
            make_identity(nc, ident)
            ident_b = singles.tile([P, P], bf16, name="ident_b")
            nc.vector.tensor_copy(out=ident_b, in_=ident)
            eps_t = singles.tile([P, 1], f32, name="eps_t")
            nc.vector.memset(eps_t, EPS)
            expb_t = singles.tile([P, 1], f32, name="expb_t")
            nc.vector.memset(expb_t, -ASHIFT)
            ones8 = singles.tile([P, 2, 32], f8, name="ones8")
            nc.vector.memset(ones8, 1.0)
            rs_sb = singles.tile([1, W], f32, name="rs_sb")
            rec_sb = singles.tile([P, IT], f32, name="rec_sb")
            aux_sb = singles.tile([P, 1040], f32, name="aux_sb")
            nc.sync.dma_start(out=aux_sb, in_=aux_d[:, :])
            s_sb = aux_sb[:, 0:512].rearrange("p (t g) -> p t g", t=CT)
            st_sb = aux_sb[:, 512:1024].rearrange("p (t c) -> p t c", t=CT)
            gsu_sb = aux_sb[:, 1024:1028]
            bp_sb = aux_sb[:, 1028:1032]
            gam_sb = aux_sb[:, 1032:1036]
            bet_sb = aux_sb[:, 1036:1040]

            # x streamed in 512-col chunks; stats overlap the DMA.
            # Tiles 0,1 stats on ACT (sum + sumsq accum), tiles 2,3 on DVE.
            for t in range(CT):
                for sg in range(NW):
                    nc.sync.dma_start(
                        out=x_sb[t][:, sg * 512:(sg + 1) * 512],
                        in_=x_d[t * P:(t + 1) * P, sg * 512:(sg + 1) * 512])
                if t == 0:
                    nc.sync.dma_start(
                        out=a8_sb,
                        in_=a8_d[:, :].rearrange("p (c s o) -> p c s o",
                                                 c=CP, s=2))
                if t == 1:
                    nc.sync.dma_start(
                        out=wpv8_sb,
                        in_=wpv8_d[:, :].rearrange("p (c s o) -> p c s o",
                                                   c=CP, s=2))

            st2_l = []
            for t in range(CT):
                st2 = gnp.tile([P, 2], f32, tag=f"st2_{t}", name=f"st2_{t}")
                st2_l.append(st2)
                if t < 2:
                    nc.scalar.activation(out=scratch, in_=x_sb[t], func=Ident,
                                         scale=1.0 / W, accum_out=st2[:, 0:1])
                    nc.scalar.activation(out=scratch, in_=x_sb[t], func=Square,
                                         scale=float(W) ** -0.5,
                                         accum_out=st2[:, 1:2])
                else:
                    stats = gnp.tile([P, NW, 6], f32, tag="bnstats",
                                     name=f"bns{t}")
                    for sg in range(NW):
                        nc.vector.bn_stats(
                            out=stats[:, sg, :],
                            in_=x_sb[t][:, sg * 512:(sg + 1) * 512])
                    mv = gnp.tile([P, 2], f32, tag="mv", name=f"mv{t}")
                    nc.vector.bn_aggr(out=mv, in_=stats)
                    nc.vector.tensor_copy(out=st2[:, 0:1], in_=mv[:, 0:1])
                    nc.vector.tensor_tensor(out=st2[:, 1:2], in0=mv[:, 0:1],
                                            in1=mv[:, 0:1], op=mult)
                    nc.vector.tensor_add(out=st2[:, 1:2], in0=st2[:, 1:2],
                                         in1=mv[:, 1:2])

            ps_a_cm = tc.tile_pool(name="ps_a", bufs=8, space="PSUM")
            ps_a = ps_a_cm.__enter__()

            for t in range(CT):
                st2 = st2_l[t]
                ps_g = ps_a.tile([P, 2], f32, tag="ps512", name=f"ps_g{t}")
                nc.tensor.matmul(ps_g[:], lhsT=s_sb[:, t, :], rhs=st2,
                                 start=True, stop=True)
                gsr = gnp.tile([P, 2], f32, tag="gsr", name=f"gsr{t}")
                nc.vector.tensor_copy(out=gsr[:8, :], in_=ps_g[:8, :])
                gs2 = gnp.tile([P, 2], f32, tag="gs2", name=f"gs2_{t}")
                nc.vector.memset(gs2, 0.0)
                nc.vector.tensor_copy(out=gs2[:8, 0:1], in_=gsr[:8, 0:1])
                nc.vector.tensor_tensor(out=gs2[:8, 1:2], in0=gsr[:8, 0:1],
                                        in1=gsr[:8, 0:1], op=mult)
                nc.vector.tensor_tensor(out=gs2[:8, 1:2], in0=gsr[:8, 1:2],
                                        in1=gs2[:8, 1:2], op=subtract)
                nc.scalar.activation(out=gs2[:8, 1:2], in_=gs2[:8, 1:2],
                                     func=Sqrt, bias=eps_t[:8], scale=1.0)
                nc.vector.reciprocal(gs2[:8, 1:2], gs2[:8, 1:2])
                ps_bc = ps_a.tile([P, 2], f32, tag="ps512", name=f"psbc{t}")
                nc.tensor.matmul(ps_bc[:], lhsT=st_sb[:, t, :],
                                 rhs=gs2, start=True, stop=True)
                bca = gnp.tile([P, 2], f32, tag="bca", name=f"bca{t}")
                nc.vector.tensor_copy(out=bca, in_=ps_bc)
                alph = gnp.tile([P, 1], f32, tag=f"alph{t}", name=f"alph{t}")
                nc.vector.tensor_tensor(out=alph, in0=bca[:, 1:2],
                                        in1=gam_sb[:, t:t + 1], op=mult)
                beta = gnp.tile([P, 1], f32, tag=f"beta{t}", name=f"beta{t}")
                nc.vector.tensor_tensor(out=beta, in0=bca[:, 0:1],
                                        in1=alph, op=mult)
                nc.vector.tensor_tensor(out=beta, in0=bet_sb[:, t:t + 1],
                                        in1=beta, op=subtract)
                h8_sl = h8[:, t // 2, t % 2, :]
                if t % 2 == 0:
                    nc.scalar.activation(out=h8_sl, in_=x_sb[t],
                                         func=Ident, scale=alph, bias=beta)
                else:
                    nc.vector.tensor_scalar(out=h8_sl, in0=x_sb[t],
                                            scalar1=alph, scalar2=beta,
                                            op0=mult, op1=add)

            # ===== u8 = fp8(Su*(A.T h + g)); copies split ACT/DVE =====
            for jc in range(NW):
                for co in range(CT):
                    ps_u = ps_a.tile([P, 512], f32, tag="ps512",
                                     name=f"psu{jc}_{co}")
                    for cp in range(CP):
                        nc.tensor.matmul(
                            ps_u[:],
                            lhsT=a8_sb[:, cp, :, co * P:(co + 1) * P],
                            rhs=h8[:, cp, :, jc * 512:(jc + 1) * 512],
                            start=(cp == 0), stop=(cp == CP - 1), perf_mode=DR)
                    u8_sl = u8[:, co // 2, co % 2, jc * 512:(jc + 1) * 512]
                    if co % 2 == 0:
                        nc.scalar.activation(
                            out=u8_sl, in_=ps_u, func=Ident,
                            scale=SU / (SA * SH), bias=gsu_sb[:, co:co + 1])
                    else:
                        nc.vector.tensor_scalar(
                            out=u8_sl, in0=ps_u, scalar1=SU / (SA * SH),
                            scalar2=gsu_sb[:, co:co + 1], op0=mult, op1=add)
            ps_a_cm.__exit__(None, None, None)

            # ===== scoresT + exp -> attT fp8 (j-major, no transposes) =====
            sc_cm = tc.tile_pool(name="ps_sc", bufs=2, space="PSUM")
            ps_sc = sc_cm.__enter__()
            for jt in range(IT):
                sc = ps_sc.tile([P, NW, 512], f32, tag="sc", name=f"sc{jt}")
                for jc in range(NW):
                    for cp in range(CP):
                        nc.tensor.matmul(
                            sc[:, jc, :],
                            lhsT=h8[:, cp, :, jt * P:(jt + 1) * P],
                            rhs=u8[:, cp, :, jc * 512:(jc + 1) * 512],
                            start=(cp == 0), stop=(cp == CP - 1), perf_mode=DR)
                nc.scalar.activation(out=att8[jt // 2][:, jt % 2, :], in_=sc,
                                     func=Exp, scale=1.0 / (SH * SU),
                                     bias=expb_t)
            sc_cm.__exit__(None, None, None)

            # ===== vp8 + row sums: PE work hidden under the exp tail =====
            ps_d_cm = tc.tile_pool(name="ps_d", bufs=1, space="PSUM")
            ps_d = ps_d_cm.__enter__()
            for jt in range(IT):
                ps_v = ps_d.tile([P, 512], f32, tag="o", bufs=4,
                                 name=f"psv{jt}")
                for cp in range(CP):
                    nc.tensor.matmul(
                        ps_v[:],
                        lhsT=h8[:, cp, :, jt * P:(jt + 1) * P],
                        rhs=wpv8_sb[:, cp, :, :],
                        start=(cp == 0), stop=(cp == CP - 1), perf_mode=DR)
                nc.vector.tensor_scalar_mul(vp8[jt // 2][:, jt % 2, :], ps_v,
                                            SVP / (SH * SWPV))
            for g in range(NW):
                ps_r = ps_d.tile([32, 512], f32, tag="rs", bufs=1,
                                 name=f"ps_r{g}")
                for jp in range(JP):
                    nc.tensor.matmul(
                        ps_r[:], lhsT=ones8[:, :, :],
                        rhs=att8[jp][:, :, g * 512:(g + 1) * 512],
                        start=(jp == 0), stop=(jp == JP - 1), perf_mode=DR)
                nc.vector.tensor_scalar_mul(rs_sb[0:1, g * 512:(g + 1) * 512],
                                            ps_r[0:1, :], SVP)
            ps_rt = ps_d.tile([P, IT], f32, tag="rst", bufs=1, name="ps_rt")
            for it in range(IT):
                nc.tensor.transpose(ps_rt[:, it:it + 1],
                                    rs_sb[0:1, it * P:(it + 1) * P],
                                    ident[0:1, 0:1])
            nc.vector.reciprocal(rec_sb, ps_rt)

            # ===== outT = attT.T @ vp (i-major); normalize; transpose back ==
            def emit_outT(it):
                ps_o = ps_d.tile([P, 512], f32, tag="o", bufs=4,
                                 name=f"ps_o{it}")
                for jp in range(JP):
                    nc.tensor.matmul(
                        ps_o[:],
                        lhsT=att8[jp][:, :, it * P:(it + 1) * P],
                        rhs=vp8[jp][:, :, :],
                        start=(jp == 0), stop=(jp == JP - 1), perf_mode=DR)
                nc.scalar.activation(out=outT[it], in_=ps_o, func=Ident,
                                     scale=rec_sb[:, it:it + 1], bias=0.0)

            def emit_final(g):
                for ot in range(CT):
                    ps_t = ps_d.tile([P, 512], bf16, tag="tr", bufs=2,
                                     name=f"ps_t{g}_{ot}")
                    for k in range(4):
                        nc.tensor.transpose(
                            ps_t[:, k * P:(k + 1) * P],
                            outT[4 * g + k][:, ot * P:(ot + 1) * P], ident_b)
                    osb = outp.tile([P, 512], f32, tag="osb", bufs=4,
                                    name=f"osb{g}_{ot}")
                    nc.vector.scalar_tensor_tensor(
                        out=osb, in0=ps_t, scalar=bp_sb[:, ot:ot + 1],
                        in1=x_sb[ot][:, g * 512:(g + 1) * 512],
                        op0=add, op1=add)
                    nc.sync.dma_start(
                        out=out_d[ot * P:(ot + 1) * P, g * 512:(g + 1) * 512],
                        in_=osb)

            for w in range(NW + 1):
                if w < NW:
                    for it in range(4 * w, 4 * w + 4):
                        emit_outT(it)
                if w >= 1:
                    emit_final(w - 1)
            ps_d_cm.__exit__(None, None, None)

    nc.finalize()
    return nc


@functools.lru_cache(maxsize=1)
def _built():
    return _build_nc()


def _pair_major(wT):
    # (C_in, C_out) -> [P, CP*2*C]: [p, cp, s, co] = wT[(2cp+s)*128+p, co]
    return np.ascontiguousarray(
        wT.reshape(CP, 2, P, C).transpose(2, 0, 1, 3).reshape(P, CP * 2 * C))


def kernel(x, gn_gamma, gn_beta, wq, bq, wk, bk, wv, bv, wp, bp):
    global LAST_EXEC_NS, LAST_TRACE_PATH
    import ml_dtypes
    from concourse.bass_utils import run_bass_kernel_spmd

    E4 = ml_dtypes.float8_e4m3
    x = np.asarray(x, dtype=np.float32)
    scale = float(C) ** -0.5
    f = np.float32
    f64 = np.float64
    wq64 = np.asarray(wq, f64)
    wk64 = np.asarray(wk, f64)
    wv64 = np.asarray(wv, f64)
    wp64 = np.asarray(wp, f64)
    # scores = h.T A h + (wk.T bq scale).h; bk terms cancel in softmax.
    # out = (wp wv h) attT; bv/bp fold through row-stochastic att into bp.
    aT = (wq64.T @ wk64 * scale).astype(f)
    wpvT = (wp64 @ wv64).T.astype(f)
    a8 = _pair_major((aT * SA).astype(E4))
    wpv8 = _pair_major((wpvT * SWPV).astype(E4))
    g_vec = (wk64.T @ (np.asarray(bq, f64) * scale)).astype(f)
    bp_eff = (np.asarray(bp, f64) + wp64 @ np.asarray(bv, f64)).astype(f)
    gam = (np.asarray(gn_gamma, f) * SH).reshape(C, 1)
    bet = (np.asarray(gn_beta, f) * SH).reshape(C, 1)

    gsz = C // G
    aux = np.zeros((P, 1040), dtype=f)
    for t in range(CT):
        for p in range(P):
            aux[p, t * P + p // gsz] = 1.0 / gsz          # S selector
            for cl in range(P):
                if p == cl // gsz:
                    aux[p, 512 + t * P + cl] = 1.0        # ST selector
    aux[:, 1024:1028] = (g_vec * SU).reshape(CT, P).T
    aux[:, 1028:1032] = bp_eff.reshape(CT, P).T
    aux[:, 1032:1036] = gam.reshape(CT, P).T
    aux[:, 1036:1040] = bet.reshape(CT, P).T

    shared = dict(a8=a8, wpv8=wpv8, aux=aux)
    in_maps = [dict(x=np.ascontiguousarray(x[i]), **shared) for i in range(B)]

    nc = _built()
    last_err = None
    for attempt in range(3):
        try:
            res = run_bass_kernel_spmd(nc, in_maps, list(range(B)), trace=TRACE)
            out = np.stack([np.asarray(res.results[i]["out"], dtype=np.float32)
                            for i in range(B)], axis=0)
            break
        except Exception as e:  # transient NRT device errors: retry
            last_err = e
            if attempt == 2:
                raise
            import time
            time.sleep(2.0)
    if TRACE:
        LAST_EXEC_NS = res.exec_time_ns
        if res.instructions_and_trace is not None:
            LAST_TRACE_PATH = res.instructions_and_trace[1]
    return out


# revision 9
# speedup vs baseline: 1.0005x; 1.0005x over previous
"""AttnBlock (GroupNorm -> QKV -> full attention -> proj + residual) on 8
Trainium2 NeuronCores, data-parallel over batch (b=8, one sample per core).

fp8 (e4m3, max 240) DoubleRow pipeline. Per core:
  h = GroupNorm(x) written directly as fp8 pairs (scale Sh folded into
  gamma/beta); x streamed in 512-col chunks with stats (sum/sumsq via
  DVE bn_stats + ACT Ident/Square accum) overlapping the DMA. Weights
  folded on host: A = (wq.T wk)/sqrt(c), WPV = wp wv. u = A.T h + g.
  scoresT[j,i] = h.T u computed DIRECTLY transposed (no PE transposes,
  no PSUM->SBUF copies): exp writes attT fp8 straight to SBUF with a
  global shift (exact softmax identity). vp and row-sum matmuls hide
  under the exp tail. outT[i,c] = attT.T @ vp is i-major so 1/rowsum is
  a per-partition activation scale; tiny PE transposes give rec. Final
  bf16 PE transpose back to c-major + fused (psum+bp)+x residual on
  DVE. All big matmuls fp8 DoubleRow (2 K-tiles per instruction).
"""

import functools

import numpy as np

B = 8
C = 512
W = 2048
G = 32
EPS = 1e-6
P = 128
CT = C // P          # 4 channel tiles
CP = CT // 2         # 2 channel-tile pairs
NW = W // 512        # 4 w-chunks of 512
IT = W // P          # 16 i-tiles
JP = IT // 2         # 8 j-tile pairs

SA = 256.0           # fp8 scale for A
SH = 16.0            # fp8 scale for h
SU = 8.0             # fp8 scale for u
SVP = 4.0            # fp8 scale for vp
SWPV = 64.0          # fp8 scale for WPV weights
ASHIFT = 1.5         # global score shift before exp (cancels in softmax)

TRACE = False
LAST_EXEC_NS = None
LAST_TRACE_PATH = None


def _build_nc():
    import concourse.mybir as mybir
    import concourse.tile as tile
    from concourse import bacc
    from concourse.masks import make_identity

    f32 = mybir.dt.float32
    f8 = mybir.dt.float8e4
    bf16 = mybir.dt.bfloat16
    Ident = mybir.ActivationFunctionType.Identity
    Exp = mybir.ActivationFunctionType.Exp
    Sqrt = mybir.ActivationFunctionType.Sqrt
    Square = mybir.ActivationFunctionType.Square
    mult = mybir.AluOpType.mult
    add = mybir.AluOpType.add
    subtract = mybir.AluOpType.subtract
    DR = mybir.MatmulPerfMode.DoubleRow

    nc = bacc.Bacc()

    x_d = nc.declare_dram_parameter("x", [C, W], f32, isOutput=False)
    # fp8 weights, pair-major [P, CP*2*C]: [p, cp, s, co] = w[(2cp+s)*128+p, co]
    a8_d = nc.declare_dram_parameter("a8", [P, CP * 2 * C], f8, isOutput=False)
    wpv8_d = nc.declare_dram_parameter("wpv8", [P, CP * 2 * C], f8, isOutput=False)
    # packed constants: [0:512] group-avg selector S, [512:1024] selector-back
    # ST, then gSu, bp_eff, gam*SH, bet*SH (CT cols each).
    aux_d = nc.declare_dram_parameter("aux", [P, 1040], f32, isOutput=False)
    out_d = nc.declare_dram_parameter("out", [C, W], f32, isOutput=True)

    with tile.TileContext(nc) as tc:
        with (
            tc.tile_pool(name="singles", bufs=1) as singles,
            tc.tile_pool(name="w8", bufs=1) as w8p,
            tc.tile_pool(name="xp", bufs=1) as xp,
            tc.tile_pool(name="h8p", bufs=1) as h8p,
            tc.tile_pool(name="att8p", bufs=1) as att8p,
            tc.tile_pool(name="outp", bufs=1) as outp,
            tc.tile_pool(name="gn", bufs=2) as gnp,
        ):
            # ---- persistent SBUF ----
            a8_sb = w8p.tile([P, CP, 2, C], f8, name="a8_sb")
            wpv8_sb = w8p.tile([P, CP, 2, C], f8, name="wpv8_sb")
            x_sb = [xp.tile([P, W], f32, name=f"x{t}") for t in range(CT)]
            h8 = h8p.tile([P, CP, 2, W], f8, name="h8")
            u8 = h8p.tile([P, CP, 2, W], f8, name="u8")
            vp8 = [h8p.tile([P, 2, C], f8, name=f"vp8_{jp}") for jp in range(JP)]
            att8 = [att8p.tile([P, 2, W], f8, name=f"att8_{jp}") for jp in range(JP)]
            outT = [outp.tile([P, 512], bf16, name=f"outT{it}") for it in range(IT)]

            ident = singles.tile([P, P], f32, name="ident")
            make_identity(nc, ident)
            ident_b = singles.tile([P, P], bf16, name="ident_b")
            nc.vector.tensor_copy(out=ident_b, in_=ident)
            eps_t = singles.tile([P, 1], f32, name="eps_t")
            nc.vector.memset(eps_t, EPS)
            expb_t = singles.tile([P, 1], f32, name="expb_t")
            nc.vector.memset(expb_t, -ASHIFT)
            ones8 = singles.tile([P, 2, 32], f8, name="ones8")
            nc.vector.memset(ones8, 1.0)
            rs_sb = singles.tile([1, W], f32, name="rs_sb")
            rec_sb = singles.tile([P, IT], f32, name="rec_sb")
            aux_sb = singles.tile([P, 1040], f32, name="aux_sb")
            nc.gpsimd.dma_start(out=aux_sb, in_=aux_d[:, :])
            s_sb = aux_sb[:, 0:512].rearrange("p (t g) -> p t g", t=CT)
            st_sb = aux_sb[:, 512:1024].rearrange("p (t c) -> p t c", t=CT)
            gsu_sb = aux_sb[:, 1024:1028]
            bp_sb = aux_sb[:, 1028:1032]
            gam_sb = aux_sb[:, 1032:1036]
            bet_sb = aux_sb[:, 1036:1040]

            # x streamed in 512-col chunks on two DMA queues (sync+gpsimd);
            # per-chunk bn_stats on DVE overlap the DMA.
            stats_l = []
            for t in range(CT):
                stats = gnp.tile([P, NW, 6], f32, tag=f"bnstats{t}",
                                 name=f"bns{t}")
                stats_l.append(stats)
            for t in range(CT):
                for sg in range(NW):
                    q = nc.sync if (t * NW + sg) % 2 == 0 else nc.gpsimd
                    q.dma_start(
                        out=x_sb[t][:, sg * 512:(sg + 1) * 512],
                        in_=x_d[t * P:(t + 1) * P, sg * 512:(sg + 1) * 512])
                    nc.vector.bn_stats(
                        out=stats_l[t][:, sg, :],
                        in_=x_sb[t][:, sg * 512:(sg + 1) * 512])
                if t == 0:
                    nc.sync.dma_start(
                        out=a8_sb,
                        in_=a8_d[:, :].rearrange("p (c s o) -> p c s o",
                                                 c=CP, s=2))
                if t == 1:
                    nc.gpsimd.dma_start(
                        out=wpv8_sb,
                        in_=wpv8_d[:, :].rearrange("p (c s o) -> p c s o",
                                                   c=CP, s=2))

            st2_l = []
            for t in range(CT):
                st2 = gnp.tile([P, 2], f32, tag=f"st2_{t}", name=f"st2_{t}")
                st2_l.append(st2)
                mv = gnp.tile([P, 2], f32, tag="mv", name=f"mv{t}")
                nc.vector.bn_aggr(out=mv, in_=stats_l[t])
                nc.vector.tensor_copy(out=st2[:, 0:1], in_=mv[:, 0:1])
                nc.vector.tensor_tensor(out=st2[:, 1:2], in0=mv[:, 0:1],
                                        in1=mv[:, 0:1], op=mult)
                nc.vector.tensor_add(out=st2[:, 1:2], in0=st2[:, 1:2],
                                     in1=mv[:, 1:2])

            ps_a_cm = tc.tile_pool(name="ps_a", bufs=8, space="PSUM")
            ps_a = ps_a_cm.__enter__()

            for t in range(CT):
                st2 = st2_l[t]
                ps_g = ps_a.tile([P, 2], f32, tag="ps512", name=f"ps_g{t}")
                nc.tensor.matmul(ps_g[:], lhsT=s_sb[:, t, :], rhs=st2,
                                 start=True, stop=True)
                gsr = gnp.tile([P, 2], f32, tag="gsr", name=f"gsr{t}")
                nc.vector.tensor_copy(out=gsr[:8, :], in_=ps_g[:8, :])
                gs2 = gnp.tile([P, 2], f32, tag="gs2", name=f"gs2_{t}")
                nc.vector.memset(gs2, 0.0)
                nc.vector.tensor_copy(out=gs2[:8, 0:1], in_=gsr[:8, 0:1])
                nc.vector.tensor_tensor(out=gs2[:8, 1:2], in0=gsr[:8, 0:1],
                                        in1=gsr[:8, 0:1], op=mult)
                nc.vector.tensor_tensor(out=gs2[:8, 1:2], in0=gsr[:8, 1:2],
                                        in1=gs2[:8, 1:2], op=subtract)
                nc.scalar.activation(out=gs2[:8, 1:2], in_=gs2[:8, 1:2],
                                     func=Sqrt, bias=eps_t[:8], scale=1.0)
                nc.vector.reciprocal(gs2[:8, 1:2], gs2[:8, 1:2])
                ps_bc = ps_a.tile([P, 2], f32, tag="ps512", name=f"psbc{t}")
                nc.tensor.matmul(ps_bc[:], lhsT=st_sb[:, t, :],
                                 rhs=gs2, start=True, stop=True)
                bca = gnp.tile([P, 2], f32, tag="bca", name=f"bca{t}")
                nc.vector.tensor_copy(out=bca, in_=ps_bc)
                alph = gnp.tile([P, 1], f32, tag=f"alph{t}", name=f"alph{t}")
                nc.vector.tensor_tensor(out=alph, in0=bca[:, 1:2],
                                        in1=gam_sb[:, t:t + 1], op=mult)
                beta = gnp.tile([P, 1], f32, tag=f"beta{t}", name=f"beta{t}")
                nc.vector.tensor_tensor(out=beta, in0=bca[:, 0:1],
                                        in1=alph, op=mult)
                nc.vector.tensor_tensor(out=beta, in0=bet_sb[:, t:t + 1],
                                        in1=beta, op=subtract)
                h8_sl = h8[:, t // 2, t % 2, :]
                if t % 2 == 0:
                    nc.scalar.activation(out=h8_sl, in_=x_sb[t],
                                         func=Ident, scale=alph, bias=beta)
                else:
                    nc.vector.tensor_scalar(out=h8_sl, in0=x_sb[t],
                                            scalar1=alph, scalar2=beta,
                                            op0=mult, op1=add)

            # ===== u8 = fp8(Su*(A.T h + g)); copies split ACT/DVE =====
            for jc in range(NW):
                for co in range(CT):
                    ps_u = ps_a.tile([P, 512], f32, tag="ps512",
                                     name=f"psu{jc}_{co}")
                    for cp in range(CP):
                        nc.tensor.matmul(
                            ps_u[:],
                            lhsT=a8_sb[:, cp, :, co * P:(co + 1) * P],
                            rhs=h8[:, cp, :, jc * 512:(jc + 1) * 512],
                            start=(cp == 0), stop=(cp == CP - 1), perf_mode=DR)
                    u8_sl = u8[:, co // 2, co % 2, jc * 512:(jc + 1) * 512]
                    if co % 2 == 0:
                        nc.scalar.activation(
                            out=u8_sl, in_=ps_u, func=Ident,
                            scale=SU / (SA * SH), bias=gsu_sb[:, co:co + 1])
                    else:
                        nc.vector.tensor_scalar(
                            out=u8_sl, in0=ps_u, scalar1=SU / (SA * SH),
                            scalar2=gsu_sb[:, co:co + 1], op0=mult, op1=add)
            ps_a_cm.__exit__(None, None, None)

            # ===== scoresT + exp -> attT fp8 (j-major, no transposes) =====
            sc_cm = tc.tile_pool(name="ps_sc", bufs=2, space="PSUM")
            ps_sc = sc_cm.__enter__()
            for jt in range(IT):
                sc = ps_sc.tile([P, NW, 512], f32, tag="sc", name=f"sc{jt}")
                for jc in range(NW):
                    for cp in range(CP):
                        nc.tensor.matmul(
                            sc[:, jc, :],
                            lhsT=h8[:, cp, :, jt * P:(jt + 1) * P],
                            rhs=u8[:, cp, :, jc * 512:(jc + 1) * 512],
                            start=(cp == 0), stop=(cp == CP - 1), perf_mode=DR)
                nc.scalar.activation(out=att8[jt // 2][:, jt % 2, :], in_=sc,
                                     func=Exp, scale=1.0 / (SH * SU),
                                     bias=expb_t)
            sc_cm.__exit__(None, None, None)

            # ===== vp8 + row sums: PE work hidden under the exp tail =====
            ps_d_cm = tc.tile_pool(name="ps_d", bufs=1, space="PSUM")
            ps_d = ps_d_cm.__enter__()
            for jt in range(IT):
                ps_v = ps_d.tile([P, 512], f32, tag="o", bufs=4,
                                 name=f"psv{jt}")
                for cp in range(CP):
                    nc.tensor.matmul(
                        ps_v[:],
                        lhsT=h8[:, cp, :, jt * P:(jt + 1) * P],
                        rhs=wpv8_sb[:, cp, :, :],
                        start=(cp == 0), stop=(cp == CP - 1), perf_mode=DR)
                nc.vector.tensor_scalar_mul(vp8[jt // 2][:, jt % 2, :], ps_v,
                                            SVP / (SH * SWPV))
            for g in range(NW):
                ps_r = ps_d.tile([32, 512], f32, tag="rs", bufs=1,
                                 name=f"ps_r{g}")
                for jp in range(JP):
                    nc.tensor.matmul(
                        ps_r[:], lhsT=ones8[:, :, :],
                        rhs=att8[jp][:, :, g * 512:(g + 1) * 512],
                        start=(jp == 0), stop=(jp == JP - 1), perf_mode=DR)
                nc.vector.tensor_scalar_mul(rs_sb[0:1, g * 512:(g + 1) * 512],
                                            ps_r[0:1, :], SVP)
            ps_rt = ps_d.tile([P, IT], f32, tag="rst", bufs=1, name="ps_rt")
            for it in range(IT):
                nc.tensor.transpose(ps_rt[:, it:it + 1],
                                    rs_sb[0:1, it * P:(it + 1) * P],
                                    ident[0:1, 0:1])
            nc.vector.reciprocal(rec_sb, ps_rt)

            # ===== outT = attT.T @ vp (i-major); normalize; transpose back ==
            def emit_outT(it):
                ps_o = ps_d.tile([P, 512], f32, tag="o", bufs=4,
                                 name=f"ps_o{it}")
                for jp in range(JP):
                    nc.tensor.matmul(
                        ps_o[:],
                        lhsT=att8[jp][:, :, it * P:(it + 1) * P],
                        rhs=vp8[jp][:, :, :],
                        start=(jp == 0), stop=(jp == JP - 1), perf_mode=DR)
                nc.scalar.activation(out=outT[it], in_=ps_o, func=Ident,
                                     scale=rec_sb[:, it:it + 1], bias=0.0)

            def emit_final(g):
                for ot in range(CT):
                    ps_t = ps_d.tile([P, 512], bf16, tag="tr", bufs=2,
                                     name=f"ps_t{g}_{ot}")
                    for k in range(4):
                        nc.tensor.transpose(
                            ps_t[:, k * P:(k + 1) * P],
                            outT[4 * g + k][:, ot * P:(ot + 1) * P], ident_b)
                    osb = outp.tile([P, 512], f32, tag="osb", bufs=4,
                                    name=f"osb{g}_{ot}")
                    nc.vector.scalar_tensor_tensor(
                        out=osb, in0=ps_t, scalar=bp_sb[:, ot:ot + 1],
                        in1=x_sb[ot][:, g * 512:(g + 1) * 512],
                        op0=add, op1=add)
                    q = nc.sync if ot % 2 == 0 else nc.gpsimd
                    q.dma_start(
                        out=out_d[ot * P:(ot + 1) * P, g * 512:(g + 1) * 512],
                        in_=osb)

            for w in range(NW + 1):
                if w < NW:
                    for it in range(4 * w, 4 * w + 4):
                        emit_outT(it)
                if w >= 1:
                    emit_final(w - 1)
            ps_d_cm.__exit__(None, None, None)

    nc.finalize()
    return nc


@functools.lru_cache(maxsize=1)
def _built():
    return _build_nc()


def _pair_major(wT):
    # (C_in, C_out) -> [P, CP*2*C]: [p, cp, s, co] = wT[(2cp+s)*128+p, co]
    return np.ascontiguousarray(
        wT.reshape(CP, 2, P, C).transpose(2, 0, 1, 3).reshape(P, CP * 2 * C))


def kernel(x, gn_gamma, gn_beta, wq, bq, wk, bk, wv, bv, wp, bp):
    global LAST_EXEC_NS, LAST_TRACE_PATH
    import ml_dtypes
    from concourse.bass_utils import run_bass_kernel_spmd

    E4 = ml_dtypes.float8_e4m3
    x = np.asarray(x, dtype=np.float32)
    scale = float(C) ** -0.5
    f = np.float32
    f64 = np.float64
    wq64 = np.asarray(wq, f64)
    wk64 = np.asarray(wk, f64)
    wv64 = np.asarray(wv, f64)
    wp64 = np.asarray(wp, f64)
    # scores = h.T A h + (wk.T bq scale).h; bk terms cancel in softmax.
    # out = (wp wv h) attT; bv/bp fold through row-stochastic att into bp.
    aT = (wq64.T @ wk64 * scale).astype(f)
    wpvT = (wp64 @ wv64).T.astype(f)
    a8 = _pair_major((aT * SA).astype(E4))
    wpv8 = _pair_major((wpvT * SWPV).astype(E4))
    g_vec = (wk64.T @ (np.asarray(bq, f64) * scale)).astype(f)
    bp_eff = (np.asarray(bp, f64) + wp64 @ np.asarray(bv, f64)).astype(f)
    gam = (np.asarray(gn_gamma, f) * SH).reshape(C, 1)
    bet = (np.asarray(gn_beta, f) * SH).reshape(C, 1)

    gsz = C // G
    aux = np.zeros((P, 1040), dtype=f)
    for t in range(CT):
        for p in range(P):
            aux[p, t * P + p // gsz] = 1.0 / gsz          # S selector
            for cl in range(P):
                if p == cl // gsz:
                    aux[p, 512 + t * P + cl] = 1.0        # ST selector
    aux[:, 1024:1028] = (g_vec * SU).reshape(CT, P).T
    aux[:, 1028:1032] = bp_eff.reshape(CT, P).T
    aux[:, 1032:1036] = gam.reshape(CT, P).T
    aux[:, 1036:1040] = bet.reshape(CT, P).T

    shared = dict(a8=a8, wpv8=wpv8, aux=aux)
    in_maps = [dict(x=np.ascontiguousarray(x[i]), **shared) for i in range(B)]

    nc = _built()
    last_err = None
    for attempt in range(3):
        try:
            res = run_bass_kernel_spmd(nc, in_maps, list(range(B)), trace=TRACE)
            out = np.stack([np.asarray(res.results[i]["out"], dtype=np.float32)
                            for i in range(B)], axis=0)
            break
        except Exception as e:  # transient NRT device errors: retry
            last_err = e
            if attempt == 2:
                raise
            import time
            time.sleep(2.0)
    if TRACE:
        LAST_EXEC_NS = res.exec_time_ns
        if res.instructions_and_trace is not None:
            LAST_TRACE_PATH = res.instructions_and_trace[1]
    return out
